# revision 29
# baseline (speedup 1.0000x reference)
"""Trainium2 Bass kernel for nn_EquivariantSHPredictor.

Strategy (pure data parallel, 8 cores, 256 samples/core, batch-major):
  front:  MLP + embedding one-hot matmuls + lift  -> H [128, 256] x2 tiles
  2 CG blocks, each per 128-sample tile:
    products:  Pr[:, (u,v)] = H[:,u] * H[:,v]  for pairs grouped by
               (m_u <= m_v, |m_u+m_v| <= 7)  -- one tensor_scalar per u
    transpose: PE transpose of s-grouped column chunks (s = m_u+m_v)
    contract:  matmul(out=t[128, w(s)], lhsT=chunkT, rhs=Tmat_chunk)
               accumulated over chunks; t is padded [128, 480]
    norm:      per (l3,c) strided sumsq -> sqrt -> recip -> scale -> compact
  out:    transpose H2, block-diag out_W matmul, transpose back, DMA out.

w3j selection rule m3 = -(m1+m2) makes each s-group contraction block-dense.
"""
import sys

sys.path.insert(0, "/opt/trn_rl_repo")

import numpy as np

import concourse.bass as bass
import concourse.bacc as bacc
import concourse.tile as tile
from concourse import mybir
from concourse.bass_utils import run_bass_kernel_spmd

L = 7
MUL = 4
NFEAT = 256
TPAD = 480
NCORES = 8
BLOCAL = 256
F32 = mybir.dt.float32
MV = list(range(-L, L + 1))


def wm(m):
    return MUL * (L + 1 - abs(m))


# ---------------- layout tables ----------------
FEAT = []
FPOS = {}
BLK_START = {}
_p = 0
for _m in MV:
    BLK_START[_m] = _p
    for _l in range(L, abs(_m) - 1, -1):
        for _i in range(MUL):
            FEAT.append((_m, _l, _i))
            FPOS[(_l, _i, _m)] = _p
            _p += 1


def tpos(l3, c, m3):
    return 32 * (m3 + L) + (L - l3) * 4 + c


def n_range(m):
    lo, hi = max(m, -L - m), min(L, L - m)
    return (lo, hi) if lo <= hi else None


PROD_OPS = []   # (u, out_col, in_col, width) one product op per u
_pc = 0
SPAN = {}
U_OUT = {}
for _m in MV:
    nr = n_range(_m)
    if nr is None:
        continue
    lo, hi = nr
    in_col = BLK_START[lo]
    width = sum(wm(n) for n in range(lo, hi + 1))
    SPAN[_m] = (in_col, width)
    for du in range(wm(_m)):
        u = BLK_START[_m] + du
        PROD_OPS.append((u, _pc, in_col, width))
        U_OUT[u] = _pc
        _pc += width
NPROD = _pc

PROD_BY_M = {}
for (u, oc, ic, w) in PROD_OPS:
    m = None
    for mv in MV:
        if BLK_START[mv] <= u < BLK_START[mv] + wm(mv):
            m = mv
            break
    PROD_BY_M.setdefault(m, []).append((u, oc, ic, w))


def _build_chunks():
    """chunks grouped by s; each chunk: product-col AP + Tmat rows."""
    out = []
    for s in range(-L, L + 1):
        chunks = []
        for m in range(max(-L, s - L), L + 1):
            n = s - m
            if 2 * m > s or n > L or n < -L:
                continue
            nr = n_range(m)
            if nr is None or not (nr[0] <= n <= nr[1]):
                continue
            wn, wmm = wm(n), wm(m)
            span_start, span_w = SPAN[m]
            n_off = BLK_START[n] - span_start
            cu_max = max(1, 128 // wn)
            du = 0
            while du < wmm:
                cu = min(cu_max, wmm - du)
                u0 = BLK_START[m] + du
                chunks.append(dict(s=s, m=m, n=n, u0=u0, cu=cu, wn=wn,
                                   col0=U_OUT[u0] + n_off, span=span_w,
                                   rows=cu * wn))
                du += cu
        out.append(dict(s=s, ws=4 * (L + 1 - abs(s)), chunks=chunks))
    return out


CHUNKS = _build_chunks()
# flat (u,v) pair-column list per s (pair-major, du-major, dv-inner) and
# simple 128-row chunk splits (chunks may span pair boundaries)
PAIRS = {}      # s -> list of (m, n, col0, w(m), w(n)) for product ops
PAIRCOLS = {}   # s -> np arrays (us, vs) of length K_s
KS = {}
PRS_MAX = 0
for _e in CHUNKS:
    _s = _e["s"]
    PAIRS[_s] = []
    _us, _vs = [], []
    _r = 0
    for _ch in _e["chunks"]:
        if not PAIRS[_s] or PAIRS[_s][-1][:2] != (_ch["m"], _ch["n"]):
            PAIRS[_s].append((_ch["m"], _ch["n"], _r, wm(_ch["m"]), _ch["wn"]))
        for _du in range(_ch["cu"]):
            for _dv in range(_ch["wn"]):
                _us.append(_ch["u0"] + _du)
                _vs.append(BLK_START[_ch["n"]] + _dv)
        _r += _ch["rows"]
    PAIRCOLS[_s] = (np.array(_us), np.array(_vs),
                    np.array([_ch for _ch in []]))
    KS[_s] = _r
    PRS_MAX = max(PRS_MAX, _r)
# diag flag per pair column (m == n) for Tmat symmetrization
PAIRDIAG = {}
for _s, _pl in PAIRS.items():
    d = np.zeros(KS[_s], bool)
    for (_m, _n, _c0, _wm, _wn) in _pl:
        if _m == _n:
            d[_c0:_c0 + _wm * _wn] = True
    PAIRDIAG[_s] = d
FLATCH = []     # (s, ws, wcol, row0, rows, toff, first, last)
_toff = 0
for _e in CHUNKS:
    _s, _ws = _e["s"], _e["ws"]
    _k = KS[_s]
    _n_ch = (_k + 127) // 128
    for _i in range(_n_ch):
        _row0 = _i * 128
        _rows = min(128, _k - _row0)
        FLATCH.append(dict(s=_s, ws=_ws, wcol=32 * (L - _s), row0=_row0,
                           rows=_rows, toff=_toff, first=(_i == 0),
                           last=(_i == _n_ch - 1)))
        _toff += _ws
TMW = _toff
NCHUNK = len(FLATCH)
PR_MAX = max(wm(m) * SPAN[m][1] for m in SPAN)


# ---------------- host constant builders ----------------
def _path_index():
    paths = {l3: [] for l3 in range(L + 1)}
    for l1 in range(L + 1):
        for l2 in range(L + 1):
            for l3 in range(abs(l1 - l2), min(l1 + l2, L) + 1):
                paths[l3].append((l1, l2))
    return paths


def _build_T(w3j, mix):
    T = np.zeros((TPAD, NFEAT, NFEAT), np.float64)
    UPOS = {l: np.array([[FPOS[(l, i, m)] for i in range(MUL)]
                         for m in range(-l, l + 1)]) for l in range(L + 1)}
    paths = _path_index()
    for l3 in range(L + 1):
        for pi, (l1, l2) in enumerate(paths[l3]):
            C = np.asarray(w3j[(l1, l2, l3)], np.float64)
            Wmix = np.asarray(mix[l3][pi * 16:(pi + 1) * 16, :], np.float64)
            m1g, m2g = np.meshgrid(np.arange(-l1, l1 + 1),
                                   np.arange(-l2, l2 + 1), indexing="ij")
            m3g = -(m1g + m2g)
            mask = np.abs(m3g) <= l3
            m1v, m2v, m3v = m1g[mask], m2g[mask], m3g[mask]
            cval = C[m1v + l1, m2v + l2, m3v + l3]
            nz = cval != 0.0
            m1v, m2v, m3v, cval = m1v[nz], m2v[nz], m3v[nz], cval[nz]
            if len(cval) == 0:
                continue
            u = UPOS[l1][m1v + l1][:, :, None, None]
            v = UPOS[l2][m2v + l2][:, None, :, None]
            w = (32 * (m3v + L) + (L - l3) * 4)[:, None, None, None] \
                + np.arange(MUL)[None, None, None, :]
            vals = cval[:, None, None, None] * Wmix.reshape(1, MUL, MUL, MUL)
            wb, ub, vb = np.broadcast_arrays(w, u, v)
            np.add.at(T, (wb.ravel(), ub.ravel(), vb.ravel()), vals.ravel())
    return T


def _build_tmat(T):
    """flat [128, TMW] fp32 contraction weights in chunk layout."""
    TM = np.zeros((128, TMW), np.float32)
    for ch in FLATCH:
        sv = ch["s"]
        wcols = np.array([32 * (L - sv) + sl for sl in range(ch["ws"])])
        us, vs = PAIRCOLS[sv][0], PAIRCOLS[sv][1]
        dg = PAIRDIAG[sv]
        sl = slice(ch["row0"], ch["row0"] + ch["rows"])
        u_s, v_s, d_s = us[sl], vs[sl], dg[sl]
        M = T[np.ix_(wcols, u_s, np.arange(1))][:, :, 0] * 0.0
        M = T[wcols[:, None], u_s[None, :], v_s[None, :]].copy()
        Msw = T[wcols[:, None], v_s[None, :], u_s[None, :]]
        M[:, ~d_s] += Msw[:, ~d_s]
        TM[:ch["rows"], ch["toff"]:ch["toff"] + ch["ws"]] = M.T.astype(np.float32)
    return TM


def _rep01():
    R = np.zeros((32, TPAD), np.float32)
    for l3 in range(L + 1):
        for c in range(MUL):
            g = l3 * 4 + c
            for m3 in range(-l3, l3 + 1):
                R[g, tpos(l3, c, m3)] = 1.0
    return R


def _build_consts(inputs):
    f32 = lambda x: np.ascontiguousarray(np.asarray(x), dtype=np.float32)
    W1, b1 = f32(inputs["mlp_W1"]), f32(inputs["mlp_b1"])
    W2, b2 = f32(inputs["mlp_W2"]), f32(inputs["mlp_b2"])
    LW = np.zeros((NFEAT, 44), np.float32)
    Lb = np.zeros((NFEAT,), np.float32)
    lift_W = [f32(w) for w in inputs["lift_W"]]
    lift_b = [f32(b) for b in inputs["lift_b"]]
    for pos, (m, l, i) in enumerate(FEAT):
        row = i * (2 * l + 1) + (m + l)
        LW[pos] = lift_W[l][row]
        Lb[pos] = lift_b[l][row]
    out_W = [f32(w) for w in inputs["out_W"]]
    G = np.zeros((NFEAT, 64), np.float32)
    col0 = np.cumsum([0] + [2 * l + 1 for l in range(L + 1)])
    for pos, (m, l, i) in enumerate(FEAT):
        G[pos, col0[l]:col0[l + 1]] = out_W[l][:, m + l] / MUL
    w3j = inputs["w3j"]
    tm, af = [], []
    for bi in range(2):
        mix = [f32(x) for x in inputs["blk_mix"][bi]]
        tm.append(_build_tmat(_build_T(w3j, mix)))
        aff = np.zeros((1, 32), np.float32)
        for l3 in range(L + 1):
            for c in range(MUL):
                aff[0, l3 * 4 + c] = f32(inputs["blk_aff"][bi][l3])[c]
        af.append(aff)
    WPACK = np.zeros((44, 68), np.float32)
    WPACK[0:25, 0:32] = W1.T
    WPACK[0:32, 32:64] = W2.T
    WPACK[0:32, 64] = b1
    WPACK[0:32, 65] = b2
    eldm = np.stack([f32(inputs["emb_lr"])[1] - f32(inputs["emb_lr"])[0],
                     f32(inputs["emb_lr"])[0]], axis=1)
    WPACK[0:4, 66:68] = eldm
    return dict(
        WPACK=WPACK,
        LWT=np.ascontiguousarray(LW.T), LB=Lb.reshape(1, NFEAT),
        EMBF=f32(inputs["emb_f"]),
        EYE=np.eye(128, dtype=np.float32),
        REP01=_rep01(), 
        TM0=tm[0], TM1=tm[1], AF0=af[0], AF1=af[1], GMAT=G,
    )


# ---------------- device program ----------------
POOL_PROD_FRAC = 0.60
DVE_COPY_FRAC = 0.30
F32R = mybir.dt.float32r   # engine per product-op index: v=vector s=scalar p=gpsimd


def _sub_ap(base_ap, col, dims):
    b = base_ap[:, col:col + 1]
    return bass.AP(tensor=b.tensor, offset=b.offset, ap=[b.ap[0]] + dims)


def _emit():
    nc = bacc.Bacc("TRN2", target_bir_lowering=False, debug=False,
                   num_devices=NCORES)
    dt = {}
    def din(name, shape):
        dt[name] = nc.dram_tensor(name, shape, F32, kind="ExternalInput")
    for name, shape in [("hx", [BLOCAL, 25]),
                        ("fidx", [1, BLOCAL]), ("eidx", [1, BLOCAL]),
                        ("WPACK", [44, 68]),
                        ("LWT", [44, NFEAT]), ("LB", [1, NFEAT]),
                        ("EMBF", [128, 8]),
                        ("EYE", [128, 128]), ("REP01", [32, TPAD]),
                        ("TM0", [128, TMW]), ("TM1", [128, TMW]),
                        ("AF0", [1, 32]), ("AF1", [1, 32]),
                        ("GMAT", [NFEAT, 64])]:
        din(name, shape)
    out_d = nc.dram_tensor("out", [BLOCAL, 64], F32, kind="ExternalOutput")

    with tile.TileContext(nc) as tc:
        import contextlib
        ctx = contextlib.ExitStack()
        with ctx:
            consts = ctx.enter_context(tc.tile_pool(name="consts", bufs=1))
            tmp = ctx.enter_context(tc.tile_pool(name="tmp", bufs=2))
            hp = ctx.enter_context(tc.tile_pool(name="hp", bufs=4))
            prp = ctx.enter_context(tc.tile_pool(name="prp", bufs=3))
            ckp = ctx.enter_context(tc.tile_pool(name="ckp", bufs=6))
            small = ctx.enter_context(tc.tile_pool(name="small", bufs=8))
            ps = ctx.enter_context(tc.tile_pool(name="ps", bufs=2, space="PSUM"))
            ps_tr = ctx.enter_context(tc.tile_pool(name="ps_tr", bufs=4, space="PSUM"))
            ps_t = ctx.enter_context(tc.tile_pool(name="ps_t", bufs=2, space="PSUM"))

            def cload(name, shape=None, engine=None):
                t = consts.tile(shape or dt[name].shape, F32, tag=name)
                (engine or nc.sync).dma_start(out=t[:], in_=dt[name][:])
                return t

            # -- critical sync-queue DMAs first (issue order = program order)
            xins = []
            for t in range(2):
                xin = tmp.tile([128, 25], F32, tag="xin")
                nc.sync.dma_start(out=xin[:], in_=dt["hx"][t * 128:(t + 1) * 128, :])
                xins.append(xin)
            fi_rep = consts.tile([128, BLOCAL], F32, tag="FIR")
            nc.sync.dma_start(out=fi_rep[:], in_=dt["fidx"][:].to_broadcast([128, BLOCAL]))
            ei4 = consts.tile([4, BLOCAL], F32, tag="EI4")
            nc.sync.dma_start(out=ei4[:], in_=dt["eidx"][:].to_broadcast([4, BLOCAL]))
            wpack = cload("WPACK")
            w1t = wpack[0:25, 0:32]
            w2t = wpack[0:32, 32:64]
            b1t = wpack[0:32, 64:65]
            b2x = wpack[0:40, 65:66]
            eld = wpack[0:4, 66:68]
            eye = cload("EYE")
            embf = cload("EMBF")
            lwt = cload("LWT")
            lwt2 = consts.tile([4, NFEAT], F32, tag="LWT2")
            nc.sync.dma_start(out=lwt2[:], in_=dt["LWT"][40:44, :])
            # -- bulk constants on the gpsimd (SWDGE) queue, needed later
            lb_rep = consts.tile([128, NFEAT], F32, tag="LBR")
            nc.gpsimd.dma_start(out=lb_rep[:], in_=dt["LB"][:].to_broadcast([128, NFEAT]))
            tm_sb = []
            af_rep = []
            for bi_ in range(2):
                t_ = consts.tile([128, TMW], F32, tag=f"TM{bi_}")
                nc.gpsimd.dma_start(out=t_[:], in_=dt[f"TM{bi_}"][:])
                tm_sb.append(t_)
                a = consts.tile([128, 32], F32, tag=f"AFR{bi_}")
                nc.gpsimd.dma_start(out=a[:], in_=dt[f"AF{bi_}"][:].to_broadcast([128, 32]))
                af_rep.append(a)
            gm0 = consts.tile([128, 64], F32, tag="GMAT0")
            nc.gpsimd.dma_start(out=gm0[:], in_=dt["GMAT"][0:128, :])
            gm1 = consts.tile([128, 64], F32, tag="GMAT1")
            nc.gpsimd.dma_start(out=gm1[:], in_=dt["GMAT"][128:256, :])
            rep01 = consts.tile([32, TPAD], F32, tag="REP01")
            nc.gpsimd.dma_start(out=rep01[:], in_=dt["REP01"][:])

            iota_i = consts.tile([128, 1], mybir.dt.int32, tag="ioi")
            nc.gpsimd.iota(iota_i[:], pattern=[[0, 1]], base=0, channel_multiplier=1)
            iota_f = consts.tile([128, 1], F32, tag="iof")
            nc.vector.tensor_copy(out=iota_f[:], in_=iota_i[:])

            # ---- front end ----
            xT = tmp.tile([25, BLOCAL], F32, tag="xT")
            for t in range(2):
                xt_ps = ps.tile([25, 128], F32, tag="st")
                nc.tensor.transpose(out=xt_ps[:], in_=xins[t][:], identity=eye[:])
                nc.scalar.copy(out=xT[:, t * 128:(t + 1) * 128], in_=xt_ps[:])
            mp = ps.tile([32, BLOCAL], F32, tag="st")
            nc.tensor.matmul(out=mp[:], lhsT=w1t, rhs=xT[:], start=True, stop=True)
            x1 = tmp.tile([32, BLOCAL], F32, tag="x1")
            nc.scalar.activation(out=x1[:], in_=mp[:],
                                 func=mybir.ActivationFunctionType.Relu, bias=b1t)
            scp = ps.tile([44, BLOCAL], F32, tag="st")
            nc.tensor.matmul(out=scp[0:32, :], lhsT=w2t, rhs=x1[:], start=True, stop=True)
            oh = tmp.tile([128, BLOCAL], F32, tag="oh")
            nc.vector.tensor_scalar(out=oh[:], in0=fi_rep[:], scalar1=iota_f[:, 0:1],
                                    scalar2=None, op0=mybir.AluOpType.is_equal)
            nc.tensor.matmul(out=scp[32:40, :], lhsT=embf[:], rhs=oh[:], start=True, stop=True)
            scT = tmp.tile([44, BLOCAL], F32, tag="scT")
            nc.scalar.activation(out=scT[0:40, :], in_=scp[0:40, :],
                                 func=mybir.ActivationFunctionType.Identity, bias=b2x)
            el4 = tmp.tile([4, BLOCAL], F32, tag="el4")
            nc.vector.tensor_scalar(out=el4[:], in0=ei4[:],
                                    scalar1=eld[:, 0:1], scalar2=eld[:, 1:2],
                                    op0=mybir.AluOpType.mult, op1=mybir.AluOpType.add)

            H = []
            for t in range(2):
                hps = ps.tile([128, NFEAT], F32, tag="st")
                nc.tensor.matmul(out=hps[:], lhsT=scT[0:40, t * 128:(t + 1) * 128],
                                 rhs=lwt[0:40, :], start=True, stop=False)
                nc.tensor.matmul(out=hps[:], lhsT=el4[:, t * 128:(t + 1) * 128],
                                 rhs=lwt2[:], start=False, stop=True)
                h = hp.tile([128, NFEAT], F32, tag="H")
                nc.vector.tensor_tensor(out=h[:], in0=hps[:], in1=lb_rep[:],
                                        op=mybir.AluOpType.add)
                H.append(h)

            # ---- CG blocks ----
            for bi in range(2):
                H2 = []
                for t in range(2):
                    h = H[t]
                    tps = ps_t.tile([128, TPAD], F32, tag="tps")
                    prod_done = 0.0
                    prod_pool = 0.0
                    copy_done = 0.0
                    copy_dve = 0.0
                    pr_of = {}
                    for e in CHUNKS:
                        s_val = e["s"]
                        if KS[s_val] == 0:
                            continue
                        pr = prp.tile([128, PRS_MAX], F32, tag="pr")
                        pr_of[s_val] = pr
                        for (m, n, c0, wmm, wn) in PAIRS[s_val]:
                            u_ap = _sub_ap(h[:], BLK_START[m], [[1, wmm], [0, wn]])
                            v_ap = _sub_ap(h[:], BLK_START[n], [[0, wmm], [1, wn]])
                            o_ap = _sub_ap(pr[:], c0, [[wn, wmm], [1, wn]])
                            w_cols = wmm * wn
                            prod_done += w_cols
                            if prod_pool < POOL_PROD_FRAC * prod_done:
                                prod_pool += w_cols
                                nc.gpsimd.tensor_tensor(out=o_ap, in0=u_ap, in1=v_ap,
                                                        op=mybir.AluOpType.mult)
                            else:
                                nc.vector.tensor_tensor(out=o_ap, in0=u_ap, in1=v_ap,
                                                        op=mybir.AluOpType.mult)
                    chs = [ch for ch in FLATCH if KS[ch["s"]]]
                    for g0 in range(0, len(chs), 4):
                        grp = chs[g0:g0 + 4]
                        tr_ps = ps_tr.tile([128, 512], F32, tag="trp")
                        for q, ch in enumerate(grp):
                            pr = pr_of[ch["s"]]
                            nc.tensor.transpose(
                                out=tr_ps[0:ch["rows"], q * 128:q * 128 + 128],
                                in_=pr[:, ch["row0"]:ch["row0"] + ch["rows"]],
                                identity=eye[:])
                        ck = ckp.tile([128, 512], F32, tag="ck")
                        wgrp = len(grp) * 128
                        copy_done += 1
                        if copy_dve < DVE_COPY_FRAC * copy_done:
                            copy_dve += 1
                            nc.vector.tensor_copy(out=ck[:, 0:wgrp], in_=tr_ps[:, 0:wgrp])
                        else:
                            nc.scalar.copy(out=ck[:, 0:wgrp], in_=tr_ps[:, 0:wgrp])
                        for q, ch in enumerate(grp):
                            nc.tensor.matmul(
                                out=tps[:, ch["wcol"]:ch["wcol"] + ch["ws"]],
                                lhsT=ck[0:ch["rows"], q * 128:q * 128 + 128],
                                rhs=tm_sb[bi][0:ch["rows"],
                                              ch["toff"]:ch["toff"] + ch["ws"]],
                                start=ch["first"], stop=ch["last"],
                                skip_group_check=True)
                    # ---- norm ----
                    tsq = tmp.tile([128, TPAD], F32, tag="tsq")
                    nc.scalar.activation(out=tsq[:], in_=tps[:],
                                         func=mybir.ActivationFunctionType.Square)
                    nsq = small.tile([128, 32], F32, tag="nsq")
                    for l3 in range(L + 1):
                        start = 32 * (L - l3) + (L - l3) * 4
                        b0 = tsq[:, start:start + 1]
                        ap = bass.AP(tensor=b0.tensor, offset=b0.offset,
                                     ap=[b0.ap[0], [1, 4], [32, 2 * l3 + 1]])
                        nc.vector.tensor_reduce(
                            out=nsq[:, l3 * 4:l3 * 4 + 4], in_=ap,
                            axis=mybir.AxisListType.X,
                            op=mybir.AluOpType.add)
                    nse = small.tile([128, 32], F32, tag="nse")
                    nc.vector.tensor_scalar_add(nse[:], nsq[:], 1e-5)
                    nrt = small.tile([128, 32], F32, tag="nrt")
                    nc.scalar.activation(out=nrt[:], in_=nse[:],
                                         func=mybir.ActivationFunctionType.Sqrt)
                    ninv = small.tile([128, 32], F32, tag="ninv")
                    nc.vector.reciprocal(out=ninv[:], in_=nrt[:])
                    ninv2 = small.tile([128, 32], F32, tag="ninv2")
                    nc.vector.tensor_tensor(out=ninv2[:], in0=ninv[:], in1=af_rep[bi][:],
                                            op=mybir.AluOpType.mult)
                    n2t_ps = ps.tile([32, 128], F32, tag="st")
                    nc.tensor.transpose(out=n2t_ps[:], in_=ninv2[:], identity=eye[:])
                    n2t = small.tile([32, 128], F32, tag="n2t")
                    nc.scalar.copy(out=n2t[:], in_=n2t_ps[:])
                    nrep_ps = ps.tile([128, TPAD], F32, tag="st")
                    nc.tensor.matmul(out=nrep_ps[:], lhsT=n2t[:], rhs=rep01[:],
                                     start=True, stop=True)
                    nrep = tmp.tile([128, TPAD], F32, tag="nrep_sb")
                    nc.scalar.copy(out=nrep[:], in_=nrep_ps[:])
                    h2p = tmp.tile([128, TPAD], F32, tag="h2p")
                    nc.vector.tensor_tensor(out=h2p[:], in0=tps[:], in1=nrep[:],
                                            op=mybir.AluOpType.mult)
                    h2 = hp.tile([128, NFEAT], F32, tag="H")
                    for m in MV:
                        bs = BLK_START[m]
                        nc.scalar.copy(out=h2[:, bs:bs + wm(m)],
                                       in_=h2p[:, 32 * (m + L):32 * (m + L) + wm(m)])
                    H2.append(h2)
                H = H2

            # ---- output ----
            h2T = []
            for half in range(2):
                hT = tmp.tile([128, BLOCAL], F32, tag=f"h2T{half}")
                for t in range(2):
                    tp2 = ps.tile([128, 128], F32, tag="st")
                    nc.tensor.transpose(out=tp2[:], in_=H[t][:, half * 128:(half + 1) * 128],
                                        identity=eye[:])
                    nc.any.tensor_copy(out=hT[:, t * 128:(t + 1) * 128], in_=tp2[:])
                h2T.append(hT)
            op_ps = ps.tile([64, BLOCAL], F32, tag="st")
            nc.tensor.matmul(out=op_ps[:], lhsT=gm0[:], rhs=h2T[0][:],
                             start=True, stop=False)
            nc.tensor.matmul(out=op_ps[:], lhsT=gm1[:], rhs=h2T[1][:],
                             start=False, stop=True)
            oT = tmp.tile([64, BLOCAL], F32, tag="oT")
            nc.vector.tensor_copy(out=oT[:], in_=op_ps[:])
            for t in range(2):
                tc_ = slice(t * 128, (t + 1) * 128)
                fo = ps.tile([128, 64], F32, tag="st")
                nc.tensor.transpose(out=fo[:], in_=oT[:, tc_],
                                    identity=eye[0:64, 0:64])
                fo_sb = tmp.tile([128, 64], F32, tag="fo_sb")
                nc.scalar.copy(out=fo_sb[:], in_=fo[:])
                nc.sync.dma_start(out=out_d[tc_, :], in_=fo_sb[:])

    nc.compile()
    return nc


_NC = None


def _in_maps(inputs):
    consts = _build_consts(inputs)
    hx = np.concatenate([np.asarray(inputs["head"], np.float32),
                         np.asarray(inputs["ear"], np.float32)], axis=1)
    hx = np.ascontiguousarray(hx)
    fi = np.asarray(inputs["freq_idx"]).astype(np.float32)
    ei = np.asarray(inputs["ear_idx"]).astype(np.float32)
    in_maps = []
    for c in range(NCORES):
        sl = slice(c * BLOCAL, (c + 1) * BLOCAL)
        m = dict(consts)
        m["hx"] = hx[sl]
        m["fidx"] = np.ascontiguousarray(fi[sl].reshape(1, BLOCAL))
        m["eidx"] = np.ascontiguousarray(ei[sl].reshape(1, BLOCAL))
        in_maps.append(m)
    return in_maps


def run_traced(**inputs):
    """Run with NTFF profiling; returns BassKernelResults (exec_time_ns etc)."""
    global _NC
    if _NC is None:
        _NC = _emit()
    return run_bass_kernel_spmd(_NC, _in_maps(inputs), list(range(NCORES)),
                                trace=True)


def kernel(**inputs):
    global _NC
    if _NC is None:
        _NC = _emit()
    res = run_bass_kernel_spmd(_NC, _in_maps(inputs), list(range(NCORES)))
    out = np.concatenate([res.results[c]["out"] for c in range(NCORES)], axis=0)
    return out.astype(np.float32)
